# revision 8
# baseline (speedup 1.0000x reference)
"""AdaFusionBlock Trainium2 kernel (8 NeuronCores, data-parallel, no collectives).

Sharding: core = b*4 + q handles batch b, output rows [32q, 32q+32).
Each core receives zero-padded input slabs (x rows +-3, y rows +-12) and
computes its output slab fully locally.

v2: pipelined. Gathers are the serial bottleneck (~6-8.5ns/idx Q7 desc-gen,
mild 4-queue overlap), so everything else is arranged to hide under the
gather stream: om conv + index prep run first, Z planes stream to DRAM with
quarter-scoped gather ranges (fine-grained deps), and the combine /
transpose / conv1 / conv2 tail is pipelined per 9-row quarter.
"""
import sys

sys.path.insert(0, "/opt/trn_rl_repo")

import numpy as np

import concourse.bass as bass
import concourse.bacc as bacc
import concourse.mybir as mybir
from concourse.tile import TileContext
from concourse.masks import make_identity

F32 = mybir.dt.float32
BF16 = mybir.dt.bfloat16
I16 = mybir.dt.int16
I32 = mybir.dt.int32
AOP = mybir.AluOpType
ACTF = mybir.ActivationFunctionType

# geometry
W = 128          # image width
WP = 130         # zero-col-padded width
C = 64           # channels
OH = 32          # output rows per core
EXT = 36         # extended out rows (+-2 halo for the two final convs)
XR = 38          # x-slab rows   [G0-3,  G0+35)
YR = 56          # y-slab rows   [G0-12, G0+44)
NK = 9           # taps
YPX = YR * W     # 7168 slab pixels
N9 = EXT * NK    # 324
NB = N9          # idx stream blocks of 128
QR = 9           # rows per quarter
NQ = 4           # quarters


def build_nc():
    nc = bacc.Bacc("TRN2", target_bir_lowering=False, num_swdge_queues=4)

    xs = nc.declare_dram_parameter("xs", [C, XR * W], BF16, isOutput=False)
    ys = nc.declare_dram_parameter("ys", [C, YR * W], BF16, isOutput=False)
    w0t = nc.declare_dram_parameter("w0t", [C, C], BF16, isOutput=False)
    b0 = nc.declare_dram_parameter("b0", [C, 1], F32, isOutput=False)
    womt = nc.declare_dram_parameter("womt", [NK * 128, 27], BF16, isOutput=False)
    bom = nc.declare_dram_parameter("bom", [27, 1], F32, isOutput=False)
    wdct = nc.declare_dram_parameter("wdct", [NK * C, C], BF16, isOutput=False)
    bdc = nc.declare_dram_parameter("bdc", [C, 1], F32, isOutput=False)
    w1t = nc.declare_dram_parameter("w1t", [NK * C, C], BF16, isOutput=False)
    b1 = nc.declare_dram_parameter("b1", [C, 1], F32, isOutput=False)
    w2t = nc.declare_dram_parameter("w2t", [NK * C, C], BF16, isOutput=False)
    b2 = nc.declare_dram_parameter("b2", [C, 1], F32, isOutput=False)
    bnd = nc.declare_dram_parameter("bnd", [128, 4], F32, isOutput=False)
    crow = nc.declare_dram_parameter("crow", [128, EXT * NK], F32, isOutput=False)
    cxw = nc.declare_dram_parameter("cxw", [128, NK], F32, isOutput=False)
    m38 = nc.declare_dram_parameter("m38", [C, XR], F32, isOutput=False)
    m36 = nc.declare_dram_parameter("m36", [C, EXT], F32, isOutput=False)
    m34 = nc.declare_dram_parameter("m34", [C, EXT - 2], F32, isOutput=False)
    outp = nc.declare_dram_parameter("out", [C, OH * W], F32, isOutput=True)

    # internal DRAM: k-grouped planes [(0,1),(2,3),(4,5),(6,7),(8,)], blocks
    # hold [kin][rowpair][64] per pixel; x-pair read via elem overlap into b+1
    NBLK = YPX + 256
    BLKG = [256, 256, 256, 256, 128]
    ZOFF = [0, 256 * NBLK, 512 * NBLK, 768 * NBLK, 1024 * NBLK]
    zp = nc.dram_tensor("zp", [1152 * NBLK], BF16)
    idxd = nc.dram_tensor("idxd", [128 * N9], I16)

    from contextlib import ExitStack

    with TileContext(nc) as tc, ExitStack() as es:
        cst = es.enter_context(tc.tile_pool(name="cst", bufs=1))
        big = es.enter_context(tc.tile_pool(name="big", bufs=1))
        ps = es.enter_context(tc.tile_pool(name="ps", bufs=2, space="PSUM"))
        pz = es.enter_context(tc.tile_pool(name="pz", bufs=1, space="PSUM"))
        pt = es.enter_context(tc.tile_pool(name="pt", bufs=1, space="PSUM"))
        gp = es.enter_context(tc.tile_pool(name="gp", bufs=2))
        sm = es.enter_context(tc.tile_pool(name="sm", bufs=1))

        # ---------- loads ----------
        xsb = gp.tile([C, XR * W], BF16, tag="xsb", name="xsb", bufs=1)
        nc.sync.dma_start(out=xsb[:, :], in_=xs[:, :])
        ysb = big.tile([C, YR * W], BF16)
        nc.sync.dma_start(out=ysb[:, :], in_=ys[:, :])

        x0y = big.tile([128, XR * WP], BF16)   # [concat-ch, XR, WP]
        nc.vector.memset(x0y[:, :], 0.0)
        x0y3 = x0y[:, :].rearrange("p (r c) -> p r c", c=WP)
        # upper half <- y rows [9, 47) of slab, into cols 1..129
        nc.sync.dma_start(
            out=bass.AP(x0y3.tensor, 64 * (XR * WP) + 1,
                        [[XR * WP, 64], [WP, XR], [1, W]]),
            in_=ys[:, :].rearrange("p (r c) -> p r c", c=W)[:, 9 : 9 + XR, :],
        )

        w0sb = cst.tile([C, C], BF16)
        nc.sync.dma_start(out=w0sb[:, :], in_=w0t[:, :])
        womsb = cst.tile([128, NK * 27], BF16)
        nc.sync.dma_start(
            out=womsb[:, :].rearrange("p (k o) -> p k o", o=27),
            in_=womt[:, :].rearrange("(k p) o -> p k o", p=128),
        )
        wdcsb = cst.tile([C, 10 * C], BF16)
        nc.vector.memset(wdcsb[:, :], 0.0)
        nc.sync.dma_start(
            out=wdcsb[:, : NK * C].rearrange("p (k o) -> p k o", o=C),
            in_=wdct[:, :].rearrange("(k p) o -> p k o", p=C),
        )
        w1sb = cst.tile([C, NK * C], BF16)
        nc.sync.dma_start(
            out=w1sb[:, :].rearrange("p (k o) -> p k o", o=C),
            in_=w1t[:, :].rearrange("(k p) o -> p k o", p=C),
        )
        w2sb = cst.tile([C, NK * C], BF16)
        nc.sync.dma_start(
            out=w2sb[:, :].rearrange("p (k o) -> p k o", o=C),
            in_=w2t[:, :].rearrange("(k p) o -> p k o", p=C),
        )
        b0sb = cst.tile([C, 1], F32)
        nc.sync.dma_start(out=b0sb[:, :], in_=b0[:, :])
        bomsb = cst.tile([27, 1], F32)
        nc.sync.dma_start(out=bomsb[:, :], in_=bom[:, :])
        bdcsb = cst.tile([C, 1], F32)
        nc.sync.dma_start(out=bdcsb[:, :], in_=bdc[:, :])
        b1sb = cst.tile([C, 1], F32)
        nc.sync.dma_start(out=b1sb[:, :], in_=b1[:, :])
        b2sb = cst.tile([C, 1], F32)
        nc.sync.dma_start(out=b2sb[:, :], in_=b2[:, :])
        bndsb = cst.tile([128, 4], F32)
        nc.sync.dma_start(out=bndsb[:, :], in_=bnd[:, :])
        crowsb = cst.tile([128, EXT * NK], F32)
        nc.sync.dma_start(out=crowsb[:, :], in_=crow[:, :])
        cxwsb = cst.tile([128, NK], F32)
        nc.sync.dma_start(out=cxwsb[:, :], in_=cxw[:, :])
        m38sb = cst.tile([C, XR], F32)
        nc.sync.dma_start(out=m38sb[:, :], in_=m38[:, :])
        m36sb = cst.tile([C, EXT], F32)
        nc.sync.dma_start(out=m36sb[:, :], in_=m36[:, :])
        m34sb = cst.tile([C, EXT - 2], F32)
        nc.sync.dma_start(out=m34sb[:, :], in_=m34[:, :])
        ident = cst.tile([128, 128], F32)
        make_identity(nc, ident[:, :])

        # ---------- conv0: x0 = W0^T x + b0 into x0y lower half ----------
        row = 0
        while row < XR:
            nr = min(4, XR - row)
            p0 = ps.tile([C, 512], F32, tag="mm", name="p0", space="PSUM")
            nc.tensor.matmul(
                p0[:, : nr * W],
                w0sb[:, :],
                xsb[:, row * W : (row + nr) * W],
                start=True, stop=True,
            )
            nc.vector.tensor_scalar(
                out=bass.AP(x0y3.tensor, row * WP + 1, [[XR * WP, C], [WP, nr], [1, W]]),
                in0=p0[:, : nr * W].rearrange("p (r c) -> p r c", c=W),
                scalar1=b0sb[:, :], scalar2=None, op0=AOP.add,
            )
            row += nr
        # zero out-of-image rows (b0 would otherwise leak into padding)
        nc.vector.tensor_tensor(
            out=x0y[0:C, :].rearrange("p (r c) -> p r c", c=WP),
            in0=x0y[0:C, :].rearrange("p (r c) -> p r c", c=WP),
            in1=bass.AP(m38sb.tensor, 0, [[XR, C], [1, XR], [0, WP]]),
            op=AOP.mult,
        )

        # ---------- om conv (9 taps, Cin=128, Cout=27) ----------
        omt = big.tile([128, EXT * 27], F32)
        omt3 = omt[:, :].rearrange("p (r o) -> p r o", o=27)
        for r0q, QRR in ((0, 8), (8, 8), (16, 8), (24, 8), (32, 4)):
            pm = ps.tile([27, 8 * W], F32, tag="mm", name="pm", space="PSUM")
            for k in range(NK):
                ki, kj = k // 3, k % 3
                n = 0
                while n < QRR:
                    nr = min(4, QRR - n)
                    re = r0q + n
                    nc.tensor.matmul(
                        pm[:, n * W : (n + nr) * W],
                        womsb[:, k * 27 : (k + 1) * 27],
                        bass.AP(x0y3.tensor, (re + ki) * WP + kj,
                                [[XR * WP, 128], [WP, nr], [1, W]]),
                        start=(k == 0), stop=(k == NK - 1),
                    )
                    n += nr
            omq = gp.tile([27, 8 * W], F32, tag="omq", name="omq")
            omq3 = omq[:, :].rearrange("p (r c) -> p r c", c=W)
            nc.vector.tensor_scalar(
                out=omq3[:, :QRR, :],
                in0=pm[:, : QRR * W].rearrange("p (r c) -> p r c", c=W),
                scalar1=bomsb[:, :], scalar2=None, op0=AOP.add,
            )
            for rq in range(QRR):
                re = r0q + rq
                ptr = pt.tile([128, 384], F32, tag="tr", name="ptr", space="PSUM")
                nc.tensor.transpose(ptr[:, :27], omq3[:, rq, :], ident[0:27, 0:27])
                nc.scalar.activation(omt3[:, re, :], ptr[:, :27], ACTF.Copy)

        # ---------- offset math (transposed layout [128, EXT, 9]) ----------
        def t9(tag):
            return sm.tile([128, N9], F32, tag=tag, name=tag)

        # offset channels are interleaved: dy_k = om[2k], dx_k = om[2k+1]
        dy = bass.AP(omt.tensor, 0, [[EXT * 27, 128], [27, EXT], [2, NK]])
        dx = bass.AP(omt.tensor, 1, [[EXT * 27, 128], [27, EXT], [2, NK]])
        mr = omt3[:, :, 18:27]

        tmp = t9("tmp")
        i32 = sm.tile([128, N9], I32, tag="i32", name="i32")
        dyf = t9("dyf")
        dxf = t9("dxf")
        # floor(x) = ((x - 0.5) + 1.5*2^23) - 1.5*2^23  (fp32 RNE magic round)
        MAGIC = 12582912.0
        nc.vector.tensor_scalar(out=tmp[:, :], in0=dy, scalar1=-0.5, scalar2=MAGIC, op0=AOP.add, op1=AOP.add)
        nc.vector.tensor_scalar(out=dyf[:, :], in0=tmp[:, :], scalar1=-MAGIC, scalar2=None, op0=AOP.add)
        nc.vector.tensor_scalar(out=tmp[:, :], in0=dx, scalar1=-0.5, scalar2=MAGIC, op0=AOP.add, op1=AOP.add)
        nc.vector.tensor_scalar(out=dxf[:, :], in0=tmp[:, :], scalar1=-MAGIC, scalar2=None, op0=AOP.add)

        ty = t9("ty")
        tx = t9("tx")
        nc.vector.tensor_tensor(out=ty[:, :], in0=dy, in1=dyf[:, :], op=AOP.subtract)
        nc.vector.tensor_tensor(out=tx[:, :], in0=dx, in1=dxf[:, :], op=AOP.subtract)
        m2 = t9("m2")
        nc.scalar.activation(m2[:, :], mr, ACTF.Sigmoid)

        r0s = t9("r0s")
        nc.vector.tensor_tensor(out=r0s[:, :], in0=crowsb[:, :], in1=dyf[:, :], op=AOP.add)
        x0g = t9("x0g")
        nc.vector.tensor_tensor(
            out=x0g[:, :],
            in0=bass.AP(cxwsb.tensor, 0, [[NK, 128], [0, EXT], [1, NK]]),
            in1=dxf[:, :].rearrange("p (r k) -> p r k", k=NK),
            op=AOP.add,
        )

        va = t9("va")
        vb = t9("vb")
        vv = t9("vv")
        p0t = t9("p0t")
        p1t = t9("p1t")
        q0t = t9("q0t")
        q1t = t9("q1t")

        def valid(src, slo, shi, dst):
            nc.vector.tensor_scalar(out=va[:, :], in0=src[:, :], scalar1=slo, scalar2=None, op0=AOP.is_ge)
            nc.vector.tensor_scalar(out=vb[:, :], in0=src[:, :], scalar1=shi, scalar2=None, op0=AOP.is_le)
            nc.vector.tensor_tensor(out=dst[:, :], in0=va[:, :], in1=vb[:, :], op=AOP.mult)

        # p0t = 2*(1-ty)*m2*vy0 ; p1t = 2*ty*m2*vy1
        valid(r0s, bndsb[:, 0:1], bndsb[:, 1:2], vv)
        nc.vector.tensor_scalar(out=p0t[:, :], in0=ty[:, :], scalar1=-2.0, scalar2=2.0, op0=AOP.mult, op1=AOP.add)
        nc.vector.tensor_tensor(out=p0t[:, :], in0=p0t[:, :], in1=m2[:, :], op=AOP.mult)
        nc.vector.tensor_tensor(out=p0t[:, :], in0=p0t[:, :], in1=vv[:, :], op=AOP.mult)
        valid(r0s, bndsb[:, 2:3], bndsb[:, 3:4], vv)
        nc.vector.tensor_scalar(out=p1t[:, :], in0=ty[:, :], scalar1=2.0, scalar2=None, op0=AOP.mult)
        nc.vector.tensor_tensor(out=p1t[:, :], in0=p1t[:, :], in1=m2[:, :], op=AOP.mult)
        nc.vector.tensor_tensor(out=p1t[:, :], in0=p1t[:, :], in1=vv[:, :], op=AOP.mult)
        # q0t = (1-tx)*vx0 ; q1t = tx*vx1
        valid(x0g, 0.0, 127.0, vv)
        nc.vector.tensor_scalar(out=q0t[:, :], in0=tx[:, :], scalar1=-1.0, scalar2=1.0, op0=AOP.mult, op1=AOP.add)
        nc.vector.tensor_tensor(out=q0t[:, :], in0=q0t[:, :], in1=vv[:, :], op=AOP.mult)
        valid(x0g, -1.0, 126.0, vv)
        nc.vector.tensor_tensor(out=q1t[:, :], in0=tx[:, :], in1=vv[:, :], op=AOP.mult)

        # U tiles bf16 (slot order: 0=(r0,x0), 1=(r1,x0), 2=(r0,x1), 3=(r1,x1))
        u = [sm.tile([128, N9], BF16, tag=f"u{j}", name=f"u{j}") for j in range(4)]
        nc.vector.tensor_tensor(out=u[0][:, :], in0=p0t[:, :], in1=q0t[:, :], op=AOP.mult)
        nc.vector.tensor_tensor(out=u[1][:, :], in0=p1t[:, :], in1=q0t[:, :], op=AOP.mult)
        nc.vector.tensor_tensor(out=u[2][:, :], in0=p0t[:, :], in1=q1t[:, :], op=AOP.mult)
        nc.vector.tensor_tensor(out=u[3][:, :], in0=p1t[:, :], in1=q1t[:, :], op=AOP.mult)

        # flat index = r0s*128 + x0g  (in-range by construction; clamp for safety)
        nc.vector.tensor_scalar(out=tmp[:, :], in0=r0s[:, :], scalar1=128.0, scalar2=None, op0=AOP.mult)
        nc.vector.tensor_tensor(out=tmp[:, :], in0=tmp[:, :], in1=x0g[:, :], op=AOP.add)
        nc.vector.tensor_scalar(out=tmp[:, :], in0=tmp[:, :], scalar1=0.0, scalar2=6920.0, op0=AOP.max, op1=AOP.min)
        nc.vector.tensor_copy(out=i32[:, :], in_=tmp[:, :])
        idx16 = sm.tile([128, N9], I16, tag="idx16", name="idx16")
        # i32 is (re, k) = (q, r, k) ordered; stream block b = (q*9 + k)*9 + r
        nc.vector.tensor_copy(
            out=bass.AP(idx16.tensor, 0, [[N9, 128], [81, NQ], [9, NK], [1, QR]]),
            in_=bass.AP(i32.tensor, 0, [[N9, 128], [81, NQ], [1, NK], [9, QR]]),
        )

        # bounce idx to DRAM (addr = lane*NB + b), then reload in the
        # dma_gather wrapped layout (replicated per 16-part group)
        nc.sync.dma_start(
            out=bass.AP(idxd, 0, [[NB, 128], [1, NB]]),
            in_=idx16[:, :],
        )
        isbpre = cst.tile([128, 8 * NB], I16)
        for g in range(8):
            nc.sync.dma_start(
                out=bass.AP(isbpre.tensor, 16 * g * (8 * NB), [[8 * NB, 16], [NB, 8], [1, NB]]),
                in_=bass.AP(idxd, 0, [[NB, 16], [16 * NB, 8], [1, NB]]),
            )
        isb = cst.tile([128, NB * 8], I16)
        nc.vector.tensor_copy(
            out=bass.AP(isb.tensor, 0, [[8 * NB, 128], [8, NB], [1, 8]]),
            in_=bass.AP(isbpre.tensor, 0, [[8 * NB, 128], [1, NB], [NB, 8]]),
        )

        # ---------- Z planes (pair-interleaved bf16, 6-slot ring, batch-3 writes) ----------
        # slot layout per partition: [g<4: kin(2) x r(2) x 64 = 256e] x4, [g4: r(2) x 64]
        ZD = 6       # ring depth
        ZB = 3       # rows per zp write batch
        zpr = big.tile([128, ZD * 2432], BF16)
        ZFS = ZD * 2432  # zpr free size (partition stride)

        def zp_write_batch(r0, nrow, s0):
            # rows [r0, r0+nrow) from zpr slots [s0, s0+nrow); one DMA per k-group
            for g in range(4):
                nc.sync.dma_start(
                    out=bass.AP(zp, ZOFF[g] + r0 * W * 256,
                                [[256, 128], [128 * 256, nrow], [1, 256]]),
                    in_=bass.AP(zpr.tensor, s0 * 2432 + g * 512,
                                [[ZFS, 128], [2432, nrow], [1, 256]]),
                )
            nc.sync.dma_start(
                out=bass.AP(zp, ZOFF[4] + r0 * W * 128, [[128, 128], [128 * 128, nrow], [1, 128]]),
                in_=bass.AP(zpr.tensor, s0 * 2432 + 2048, [[ZFS, 128], [2432, nrow], [1, 128]]),
            )

        for ch in range(YR):
            pzt = pz.tile([128, 10 * C], F32, tag="pz", name="pzt", space="PSUM")
            nc.tensor.matmul(
                pzt[:, 0 : 8 * C],
                ysb[:, ch * W : (ch + 1) * W],
                wdcsb[:, 0 : 8 * C],
                start=True, stop=True,
            )
            nc.tensor.matmul(
                pzt[:, 8 * C : 10 * C],
                ysb[:, ch * W : (ch + 1) * W],
                wdcsb[:, 8 * C : 10 * C],
                start=True, stop=True,
            )
            s = ch % ZD
            # slot r0 (this row): all 10 slots in one strided copy
            nc.scalar.activation(
                bass.AP(zpr.tensor, s * 2432, [[ZFS, 128], [512, 5], [128, 2], [1, C]]),
                pzt[:, :].rearrange("p (g j o) -> p g j o", j=2, o=C),
                ACTF.Copy,
            )
            # slot r1 into previous row's slot (offset +64)
            if ch > 0:
                sp = (ch - 1) % ZD
                nc.vector.tensor_copy(
                    out=bass.AP(zpr.tensor, sp * 2432 + C, [[ZFS, 128], [512, 5], [128, 2], [1, C]]),
                    in_=pzt[:, :].rearrange("p (g j o) -> p g j o", j=2, o=C),
                )
                # batch-write fully completed rows [ch-ZB .. ch-1] when aligned
                if ch % ZB == 0 and ch >= ZB:
                    zp_write_batch(ch - ZB, ZB, (ch - ZB) % ZD)
        # tail rows: 54, 55 (55's r1 is garbage; never addressed by valid idx)
        zp_write_batch(YR - 2, 2, (YR - 2) % ZD)

        # ---------- gather + combine + per-quarter tail ----------
        oslab = big.tile([C, EXT * WP], BF16)
        os3 = oslab[:, :].rearrange("p (r c) -> p r c", c=WP)
        nc.vector.memset(oslab[:, :], 0.0)
        t1 = big.tile([C, (EXT - 2) * WP], BF16)
        t13 = t1[:, :].rearrange("p (r c) -> p r c", c=WP)
        nc.vector.memset(t1[:, :], 0.0)

        # per-quarter conv1 chunks (EXT row ranges) and conv2 chunks (out rows)
        C1CH = [[(1, 7)], [(8, 8), (16, 1)], [(17, 8), (25, 1)], [(26, 8), (34, 1)]]
        C2CH = [[(0, 5)], [(5, 8), (13, 1)], [(14, 8), (22, 1)], [(23, 8), (31, 1)]]

        qn = 0
        for q in range(NQ):
            CNT = (QR * q + 28) * 128  # zp blocks addressable by this quarter
            sacc = [gp.tile([128, QR * C], BF16, tag=f"sacc{j}", name=f"sacc{q}_{j}", bufs=2)
                    for j in range(4)]
            for k in range(NK):
                g4, kin = k // 2, k % 2
                blk = BLKG[g4]
                esz = blk + 128
                soff = [0, C, blk, blk + C]
                grun = gp.tile([128, QR * 384], BF16, tag="grun", name="grun", bufs=3)
                gv = grun[:, : QR * esz].rearrange("p (r e) -> p r e", e=esz)
                base = (q * NK + k) * QR  # stream block offset
                # split 1152 idx into 1024+128 (num_idxs > 1024 faults on HW)
                for b0, nblk in ((0, 8), (8, 1)):
                    nc.gpsimd.dma_gather(
                        gv[:, b0 : b0 + nblk, :],
                        bass.AP(zp, ZOFF[g4] + kin * 128, [[blk, CNT], [1, esz]]),
                        isb[:, (base + b0) * 8 : (base + b0 + nblk) * 8],
                        num_idxs=nblk * 128,
                        num_idxs_reg=nblk * 128,
                        elem_size=esz,
                        elem_step=blk,
                        queue_num=qn % 4,
                    )
                    qn += 1
                tmpc = gp.tile([128, QR * C], BF16, tag="tmpc", name="tmpc", bufs=2)
                for j in range(4):
                    uap = bass.AP(u[j].tensor, 81 * q + k, [[N9, 128], [NK, QR], [0, C]])
                    if k == 0:
                        nc.vector.tensor_tensor(
                            out=sacc[j][:, :].rearrange("p (r o) -> p r o", o=C),
                            in0=gv[:, :, soff[j] : soff[j] + C],
                            in1=uap,
                            op=AOP.mult,
                        )
                    else:
                        nc.vector.tensor_tensor(
                            out=tmpc[:, :].rearrange("p (r o) -> p r o", o=C),
                            in0=gv[:, :, soff[j] : soff[j] + C],
                            in1=uap,
                            op=AOP.mult,
                        )
                        nc.vector.tensor_tensor(
                            out=sacc[j][:, :], in0=sacc[j][:, :], in1=tmpc[:, :], op=AOP.add
                        )
            # fold 4 corner chains -> f32 acc
            accq = gp.tile([128, QR * C], F32, tag="accq", name=f"accq{q}", bufs=2)
            acc2 = gp.tile([128, QR * C], F32, tag="acc2", name=f"acc2{q}", bufs=2)
            nc.vector.tensor_tensor(out=accq[:, :], in0=sacc[0][:, :], in1=sacc[1][:, :], op=AOP.add)
            nc.vector.tensor_tensor(out=acc2[:, :], in0=sacc[2][:, :], in1=sacc[3][:, :], op=AOP.add)
            nc.vector.tensor_tensor(out=accq[:, :], in0=accq[:, :], in1=acc2[:, :], op=AOP.add)
            acc3 = accq[:, :].rearrange("p (r o) -> p r o", o=C)

            # transpose back (3 batches of 3 rows) + bdc + x0 residual -> oslab
            for rb in range(3):
                ptb = pt.tile([128, 384], F32, tag="tr", name="ptb", space="PSUM")
                for i in range(3):
                    nc.tensor.transpose(
                        ptb[0:C, i * W : (i + 1) * W],
                        acc3[:, rb * 3 + i, :],
                        ident[:, :],
                    )
                tdc = gp.tile([C, 384], F32, tag="tdc", bufs=2)
                nc.vector.tensor_scalar(out=tdc[:, :], in0=ptb[0:C, :384], scalar1=bdcsb[:, :], scalar2=None, op0=AOP.add)
                re0 = q * QR + rb * 3
                nc.vector.tensor_tensor(
                    out=bass.AP(os3.tensor, re0 * WP + 1, [[EXT * WP, C], [WP, 3], [1, W]]),
                    in0=tdc[:, :].rearrange("p (r c) -> p r c", c=W),
                    in1=bass.AP(x0y3.tensor, (re0 + 1) * WP + 1, [[XR * WP, C], [WP, 3], [1, W]]),
                    op=AOP.add,
                )
            # zero out-of-image rows for this quarter
            nc.vector.tensor_tensor(
                out=os3[:, q * QR : (q + 1) * QR, :],
                in0=os3[:, q * QR : (q + 1) * QR, :],
                in1=bass.AP(m36sb.tensor, q * QR, [[EXT, C], [1, QR], [0, WP]]),
                op=AOP.mult,
            )

            # conv1 chunks now enabled by this quarter (+ lrelu + m34 mask)
            for r0c, nr_h in C1CH[q]:
                pc1 = ps.tile([C, 8 * W], F32, tag="mm", name="pc1", space="PSUM")[:, : nr_h * W]
                for k in range(NK):
                    ki, kj = k // 3, k % 3
                    n = 0
                    while n < nr_h:
                        nr = min(4, nr_h - n)
                        re = r0c + n
                        nc.tensor.matmul(
                            pc1[:, n * W : (n + nr) * W],
                            w1sb[:, k * C : (k + 1) * C],
                            bass.AP(os3.tensor, (re - 1 + ki) * WP + kj, [[EXT * WP, C], [WP, nr], [1, W]]),
                            start=(k == 0), stop=(k == NK - 1),
                        )
                        n += nr
                tl = gp.tile([C, 8 * W], F32, tag="tl", name="tl", bufs=2)
                nc.vector.tensor_scalar(out=tl[:, : nr_h * W], in0=pc1[:, :], scalar1=b1sb[:, :], scalar2=None, op0=AOP.add)
                # t1 row (T1 coords = EXT row - 1) = lrelu * rowmask
                nc.vector.scalar_tensor_tensor(
                    out=bass.AP(t13.tensor, (r0c - 1) * WP + 1, [[(EXT - 2) * WP, C], [WP, nr_h], [1, W]]),
                    in0=tl[:, : nr_h * W].rearrange("p (r c) -> p r c", c=W),
                    scalar=0.2,
                    in1=tl[:, : nr_h * W].rearrange("p (r c) -> p r c", c=W),
                    op0=AOP.mult,
                    op1=AOP.max,
                )
                nc.vector.tensor_tensor(
                    out=t13[:, r0c - 1 : r0c - 1 + nr_h, :],
                    in0=t13[:, r0c - 1 : r0c - 1 + nr_h, :],
                    in1=bass.AP(m34sb.tensor, r0c - 1, [[EXT - 2, C], [1, nr_h], [0, WP]]),
                    op=AOP.mult,
                )

            # conv2 chunks + residual + store
            for o0, nr_h in C2CH[q]:
                pc2 = ps.tile([C, 8 * W], F32, tag="mm", name="pc2", space="PSUM")[:, : nr_h * W]
                for k in range(NK):
                    ki, kj = k // 3, k % 3
                    n = 0
                    while n < nr_h:
                        nr = min(4, nr_h - n)
                        # conv2 out row o -> t1 rows (o+2-1+ki)-1 = o+ki in T1 coords
                        nc.tensor.matmul(
                            pc2[:, n * W : (n + nr) * W],
                            w2sb[:, k * C : (k + 1) * C],
                            bass.AP(t13.tensor, (o0 + n + ki) * WP + kj, [[(EXT - 2) * WP, C], [WP, nr], [1, W]]),
                            start=(k == 0), stop=(k == NK - 1),
                        )
                        n += nr
                tf = gp.tile([C, 8 * W], F32, tag="tf", name="tf", bufs=2)
                nc.vector.tensor_scalar(out=tf[:, : nr_h * W], in0=pc2[:, :], scalar1=b2sb[:, :], scalar2=None, op0=AOP.add)
                nc.vector.tensor_tensor(
                    out=tf[:, : nr_h * W].rearrange("p (r c) -> p r c", c=W),
                    in0=tf[:, : nr_h * W].rearrange("p (r c) -> p r c", c=W),
                    in1=bass.AP(os3.tensor, (o0 + 2) * WP + 1, [[EXT * WP, C], [WP, nr_h], [1, W]]),
                    op=AOP.add,
                )
                nc.sync.dma_start(
                    out=outp[:, o0 * W : (o0 + nr_h) * W], in_=tf[:, : nr_h * W]
                )

    nc.finalize()
    return nc


# ---------------- host side ----------------

_NC_CACHE = None


def _get_nc():
    global _NC_CACHE
    if _NC_CACHE is None:
        _NC_CACHE = build_nc()
    return _NC_CACHE


def _prep_core(inputs, b, q):
    G0 = 32 * q
    x = inputs["x"][b]  # [64, 128, 128]
    y = inputs["y"][b]

    def slab(img, lo, rows):
        out = np.zeros((C, rows, W), np.float32)
        for i in range(rows):
            g = lo + i
            if 0 <= g < 128:
                out[:, i, :] = img[:, g, :]
        return out

    import ml_dtypes
    bf = ml_dtypes.bfloat16
    xs = slab(x, G0 - 3, XR).reshape(C, XR * W).astype(bf)
    ysl = slab(y, G0 - 12, YR).reshape(C, YR * W).astype(bf)

    w0t = inputs["w0"][:, :, 0, 0].T.copy().astype(bf)  # [c, o]
    womt = (np.transpose(inputs["w_om"], (2, 3, 1, 0)).reshape(NK, 128, 27).reshape(NK * 128, 27).copy()).astype(bf)
    wdct = (np.transpose(inputs["w_dc"], (2, 3, 1, 0)).reshape(NK, C, C).reshape(NK * C, C).copy()).astype(bf)
    w1t = (np.transpose(inputs["w1"], (2, 3, 1, 0)).reshape(NK, C, C).reshape(NK * C, C).copy()).astype(bf)
    w2t = (np.transpose(inputs["w2"], (2, 3, 1, 0)).reshape(NK, C, C).reshape(NK * C, C).copy()).astype(bf)

    lo = 12.0 - G0
    hi = 139.0 - G0
    bnd = np.tile(np.array([[lo, hi, lo - 1.0, hi - 1.0]], np.float32), (128, 1))

    re_idx = np.arange(EXT)[:, None]
    ki = (np.arange(NK) // 3)[None, :]
    kj = (np.arange(NK) % 3)[None, :]
    crow_row = (re_idx + ki + 9).astype(np.float32).reshape(1, EXT * NK)
    crow = np.tile(crow_row, (128, 1))
    wv = np.arange(128)[:, None].astype(np.float32)
    cxw = (wv - 1.0 + kj.astype(np.float32))  # [128, 9]

    def rowmask(lo_r, rows):
        g = lo_r + np.arange(rows)
        m = ((g >= 0) & (g < 128)).astype(np.float32)
        return np.tile(m[None, :], (C, 1))

    return {
        "xs": xs,
        "ys": ysl,
        "w0t": w0t,
        "b0": inputs["b0"].reshape(C, 1).astype(np.float32),
        "womt": womt,
        "bom": inputs["b_om"].reshape(27, 1).astype(np.float32),
        "wdct": wdct,
        "bdc": inputs["b_dc"].reshape(C, 1).astype(np.float32),
        "w1t": w1t,
        "b1": inputs["b1"].reshape(C, 1).astype(np.float32),
        "w2t": w2t,
        "b2": inputs["b2"].reshape(C, 1).astype(np.float32),
        "bnd": bnd,
        "crow": crow,
        "cxw": cxw.astype(np.float32),
        "m38": rowmask(G0 - 3, XR),
        "m36": rowmask(G0 - 2, EXT),
        "m34": rowmask(G0 - 1, EXT - 2),
    }


def make_in_maps(inputs):
    inputs = {k: np.asarray(v, np.float32) for k, v in inputs.items()}
    return [_prep_core(inputs, core // 4, core % 4) for core in range(8)]


def kernel(**inputs):
    from concourse.bass_utils import run_bass_kernel_spmd

    nc = _get_nc()
    in_maps = make_in_maps(inputs)
    res = run_bass_kernel_spmd(nc, in_maps, core_ids=list(range(8)))
    out = np.zeros((2, C, 128, W), np.float32)
    for core in range(8):
        b, q = core // 4, core % 4
        out[b, :, 32 * q : 32 * q + 32, :] = res.results[core]["out"].reshape(C, OH, W)
    return out


# revision 11
# speedup vs baseline: 1.2956x; 1.2956x over previous
"""AdaFusionBlock Trainium2 kernel (8 NeuronCores, data-parallel, no collectives).

Sharding: core = b*4 + q handles batch b, output rows [32q, 32q+32).
Each core receives zero-padded input slabs (x rows +-3, y rows +-12) and
computes its output slab fully locally.

v4: pipelined around the Q7 gather wall (~6-8.4ns/idx desc-gen, mild
4-queue overlap). Structure:
  - conv0 + om conv + offset/index math run first (PE/Vector/Scalar),
    index bounce DMAs on the GpSimd queue (which then runs the gathers).
  - Z planes stream to DRAM; zp write batches alternate Sync/Scalar HWDGE
    queues; gather in_aps are range-scoped per row-group so gathers can
    start before the full Z phase completes.
  - Gathers: 5 row-groups (8,8,8,8,4 rows) x 9 taps, <=1024 idx per call,
    queues rotating 0..3 per call.
  - Combine uses paired-corner ops ((r0,r1) x 64ch contiguous in the
    gathered block) to halve DVE instruction count; per-group fold /
    transpose / conv1 / conv2 / store pipeline behind the gather stream.
"""
import sys

sys.path.insert(0, "/opt/trn_rl_repo")

import numpy as np

import concourse.bass as bass
import concourse.bacc as bacc
import concourse.mybir as mybir
from concourse.tile import TileContext
from concourse.masks import make_identity

F32 = mybir.dt.float32
BF16 = mybir.dt.bfloat16
I16 = mybir.dt.int16
I32 = mybir.dt.int32
AOP = mybir.AluOpType
ACTF = mybir.ActivationFunctionType

# geometry
W = 128          # image width
WP = 130         # zero-col-padded width
C = 64           # channels
OH = 32          # output rows per core
EXT = 36         # extended out rows (+-2 halo for the two final convs)
XR = 38          # x-slab rows   [G0-3,  G0+35)
YR = 56          # y-slab rows   [G0-12, G0+44)
NK = 9           # taps
YPX = YR * W     # 7168 slab pixels
N9 = EXT * NK    # 324
NB = N9          # idx stream blocks of 128
GRP = [(0, 8), (8, 8), (16, 8), (24, 8), (32, 4)]  # (row base, rows)
C1CH = [(1, 6), (7, 8), (15, 8), (23, 8), (31, 4)]   # conv1 EXT-row chunks
C2CH = [(0, 4), (4, 8), (12, 8), (20, 8), (28, 4)]   # conv2 out-row chunks


def build_nc():
    nc = bacc.Bacc("TRN2", target_bir_lowering=False, num_swdge_queues=4)

    xs = nc.declare_dram_parameter("xs", [C, XR * W], BF16, isOutput=False)
    ys = nc.declare_dram_parameter("ys", [C, YR * W], BF16, isOutput=False)
    w0t = nc.declare_dram_parameter("w0t", [C, C], BF16, isOutput=False)
    b0 = nc.declare_dram_parameter("b0", [C, 1], F32, isOutput=False)
    womt = nc.declare_dram_parameter("womt", [NK * 128, 27], BF16, isOutput=False)
    bom = nc.declare_dram_parameter("bom", [27, 1], F32, isOutput=False)
    wdct = nc.declare_dram_parameter("wdct", [NK * C, C], BF16, isOutput=False)
    bdc = nc.declare_dram_parameter("bdc", [C, 1], F32, isOutput=False)
    w1t = nc.declare_dram_parameter("w1t", [NK * C, C], BF16, isOutput=False)
    b1 = nc.declare_dram_parameter("b1", [C, 1], F32, isOutput=False)
    w2t = nc.declare_dram_parameter("w2t", [NK * C, C], BF16, isOutput=False)
    b2 = nc.declare_dram_parameter("b2", [C, 1], F32, isOutput=False)
    bnd = nc.declare_dram_parameter("bnd", [128, 4], F32, isOutput=False)
    crow = nc.declare_dram_parameter("crow", [128, EXT * NK], F32, isOutput=False)
    cxw = nc.declare_dram_parameter("cxw", [128, NK], F32, isOutput=False)
    m38 = nc.declare_dram_parameter("m38", [C, XR], F32, isOutput=False)
    m36 = nc.declare_dram_parameter("m36", [C, EXT], F32, isOutput=False)
    m34 = nc.declare_dram_parameter("m34", [C, EXT - 2], F32, isOutput=False)
    outp = nc.declare_dram_parameter("out", [C, OH * W], F32, isOutput=True)

    # internal DRAM: k-grouped planes [(0,1),(2,3),(4,5),(6,7),(8,)], blocks
    # hold [kin][rowpair][64] per pixel; x-pair read via elem overlap into b+1
    NBLK = YPX + 256
    BLKG = [256, 256, 256, 256, 128]
    ZOFF = [0, 256 * NBLK, 512 * NBLK, 768 * NBLK, 1024 * NBLK]
    zp = nc.dram_tensor("zp", [1152 * NBLK], BF16)
    idxd = nc.dram_tensor("idxd", [128 * N9], I16)

    from contextlib import ExitStack

    with TileContext(nc) as tc, ExitStack() as es:
        cst = es.enter_context(tc.tile_pool(name="cst", bufs=1))
        big = es.enter_context(tc.tile_pool(name="big", bufs=1))
        ps = es.enter_context(tc.tile_pool(name="ps", bufs=2, space="PSUM"))
        pz = es.enter_context(tc.tile_pool(name="pz", bufs=1, space="PSUM"))
        pt = es.enter_context(tc.tile_pool(name="pt", bufs=1, space="PSUM"))
        gp = es.enter_context(tc.tile_pool(name="gp", bufs=2))
        sm = es.enter_context(tc.tile_pool(name="sm", bufs=1))

        # ---------- loads (inputs on Sync; weights/consts on Scalar HWDGE) ----------
        xsb = gp.tile([C, XR * W], BF16, tag="xsb", name="xsb", bufs=1)
        nc.sync.dma_start(out=xsb[:, :], in_=xs[:, :])
        ysb = big.tile([C, YR * W], BF16)
        nc.sync.dma_start(out=ysb[:, :], in_=ys[:, :])

        x0y = big.tile([128, XR * WP], BF16)   # [concat-ch, XR, WP]
        nc.vector.memset(x0y[:, :], 0.0)
        x0y3 = x0y[:, :].rearrange("p (r c) -> p r c", c=WP)
        # upper half <- y rows [9, 47) of slab, into cols 1..129
        nc.sync.dma_start(
            out=bass.AP(x0y3.tensor, 64 * (XR * WP) + 1,
                        [[XR * WP, 64], [WP, XR], [1, W]]),
            in_=ys[:, :].rearrange("p (r c) -> p r c", c=W)[:, 9 : 9 + XR, :],
        )

        w0sb = cst.tile([C, C], BF16)
        nc.scalar.dma_start(out=w0sb[:, :], in_=w0t[:, :])
        womsb = cst.tile([128, NK * 27], BF16)
        nc.scalar.dma_start(
            out=womsb[:, :].rearrange("p (k o) -> p k o", o=27),
            in_=womt[:, :].rearrange("(k p) o -> p k o", p=128),
        )
        wdcsb = cst.tile([C, 10 * C], BF16)
        nc.vector.memset(wdcsb[:, :], 0.0)
        nc.scalar.dma_start(
            out=wdcsb[:, : NK * C].rearrange("p (k o) -> p k o", o=C),
            in_=wdct[:, :].rearrange("(k p) o -> p k o", p=C),
        )
        w1sb = cst.tile([C, NK * C], BF16)
        nc.scalar.dma_start(
            out=w1sb[:, :].rearrange("p (k o) -> p k o", o=C),
            in_=w1t[:, :].rearrange("(k p) o -> p k o", p=C),
        )
        w2sb = cst.tile([C, NK * C], BF16)
        nc.scalar.dma_start(
            out=w2sb[:, :].rearrange("p (k o) -> p k o", o=C),
            in_=w2t[:, :].rearrange("(k p) o -> p k o", p=C),
        )
        b0sb = cst.tile([C, 1], F32)
        nc.scalar.dma_start(out=b0sb[:, :], in_=b0[:, :])
        bomsb = cst.tile([27, 1], F32)
        nc.scalar.dma_start(out=bomsb[:, :], in_=bom[:, :])
        bdcsb = cst.tile([C, 1], F32)
        nc.scalar.dma_start(out=bdcsb[:, :], in_=bdc[:, :])
        b1sb = cst.tile([C, 1], F32)
        nc.scalar.dma_start(out=b1sb[:, :], in_=b1[:, :])
        b2sb = cst.tile([C, 1], F32)
        nc.scalar.dma_start(out=b2sb[:, :], in_=b2[:, :])
        bndsb = cst.tile([128, 4], F32)
        nc.scalar.dma_start(out=bndsb[:, :], in_=bnd[:, :])
        crowsb = cst.tile([128, EXT * NK], F32)
        nc.scalar.dma_start(out=crowsb[:, :], in_=crow[:, :])
        cxwsb = cst.tile([128, NK], F32)
        nc.scalar.dma_start(out=cxwsb[:, :], in_=cxw[:, :])
        m38sb = cst.tile([C, XR], F32)
        nc.scalar.dma_start(out=m38sb[:, :], in_=m38[:, :])
        m36sb = cst.tile([C, EXT], F32)
        nc.scalar.dma_start(out=m36sb[:, :], in_=m36[:, :])
        m34sb = cst.tile([C, EXT - 2], F32)
        nc.scalar.dma_start(out=m34sb[:, :], in_=m34[:, :])
        ident = cst.tile([128, 128], F32)
        make_identity(nc, ident[:, :])

        # ---------- conv0: x0 = W0^T x + b0 into x0y lower half ----------
        row = 0
        while row < XR:
            nr = min(4, XR - row)
            p0 = ps.tile([C, 512], F32, tag="mm", name="p0", space="PSUM")
            nc.tensor.matmul(
                p0[:, : nr * W],
                w0sb[:, :],
                xsb[:, row * W : (row + nr) * W],
                start=True, stop=True,
            )
            nc.vector.tensor_scalar(
                out=bass.AP(x0y3.tensor, row * WP + 1, [[XR * WP, C], [WP, nr], [1, W]]),
                in0=p0[:, : nr * W].rearrange("p (r c) -> p r c", c=W),
                scalar1=b0sb[:, :], scalar2=None, op0=AOP.add,
            )
            row += nr
        # zero out-of-image rows (b0 would otherwise leak into padding)
        nc.vector.tensor_tensor(
            out=x0y[0:C, :].rearrange("p (r c) -> p r c", c=WP),
            in0=x0y[0:C, :].rearrange("p (r c) -> p r c", c=WP),
            in1=bass.AP(m38sb.tensor, 0, [[XR, C], [1, XR], [0, WP]]),
            op=AOP.mult,
        )

        # ---------- om conv (9 taps, Cin=128, Cout=27) ----------
        omt = big.tile([128, EXT * 27], F32)
        omt3 = omt[:, :].rearrange("p (r o) -> p r o", o=27)
        for r0q, QRR in ((0, 8), (8, 8), (16, 8), (24, 8), (32, 4)):
            pm = ps.tile([27, 8 * W], F32, tag="mm", name="pm", space="PSUM")
            for k in range(NK):
                ki, kj = k // 3, k % 3
                n = 0
                while n < QRR:
                    nr = min(4, QRR - n)
                    re = r0q + n
                    nc.tensor.matmul(
                        pm[:, n * W : (n + nr) * W],
                        womsb[:, k * 27 : (k + 1) * 27],
                        bass.AP(x0y3.tensor, (re + ki) * WP + kj,
                                [[XR * WP, 128], [WP, nr], [1, W]]),
                        start=(k == 0), stop=(k == NK - 1),
                    )
                    n += nr
            omq = gp.tile([27, 8 * W], F32, tag="omq", name="omq")
            omq3 = omq[:, :].rearrange("p (r c) -> p r c", c=W)
            nc.vector.tensor_scalar(
                out=omq3[:, :QRR, :],
                in0=pm[:, : QRR * W].rearrange("p (r c) -> p r c", c=W),
                scalar1=bomsb[:, :], scalar2=None, op0=AOP.add,
            )
            for rq in range(QRR):
                re = r0q + rq
                ptr = pt.tile([128, 512], F32, tag="tr", name="ptr", space="PSUM")
                nc.tensor.transpose(ptr[:, :27], omq3[:, rq, :], ident[0:27, 0:27])
                nc.scalar.activation(omt3[:, re, :], ptr[:, :27], ACTF.Copy)

        # ---------- offset math (transposed layout [128, EXT, 9]) ----------
        def t9(tag):
            return sm.tile([128, N9], F32, tag=tag, name=tag)

        # offset channels are interleaved: dy_k = om[2k], dx_k = om[2k+1]
        dy = bass.AP(omt.tensor, 0, [[EXT * 27, 128], [27, EXT], [2, NK]])
        dx = bass.AP(omt.tensor, 1, [[EXT * 27, 128], [27, EXT], [2, NK]])
        mr = omt3[:, :, 18:27]

        tmp = t9("tmp")
        i32 = sm.tile([128, N9], I32, tag="i32", name="i32")
        dyf = t9("dyf")
        dxf = t9("dxf")
        # floor(x) = ((x - 0.5) + 1.5*2^23) - 1.5*2^23  (fp32 RNE magic round)
        MAGIC = 12582912.0
        nc.vector.tensor_scalar(out=tmp[:, :], in0=dy, scalar1=-0.5, scalar2=MAGIC, op0=AOP.add, op1=AOP.add)
        nc.vector.tensor_scalar(out=dyf[:, :], in0=tmp[:, :], scalar1=-MAGIC, scalar2=None, op0=AOP.add)
        nc.vector.tensor_scalar(out=tmp[:, :], in0=dx, scalar1=-0.5, scalar2=MAGIC, op0=AOP.add, op1=AOP.add)
        nc.vector.tensor_scalar(out=dxf[:, :], in0=tmp[:, :], scalar1=-MAGIC, scalar2=None, op0=AOP.add)

        ty = t9("ty")
        tx = t9("tx")
        nc.vector.tensor_tensor(out=ty[:, :], in0=dy, in1=dyf[:, :], op=AOP.subtract)
        nc.vector.tensor_tensor(out=tx[:, :], in0=dx, in1=dxf[:, :], op=AOP.subtract)
        m2 = t9("m2")
        nc.scalar.activation(m2[:, :], mr, ACTF.Sigmoid)

        r0s = t9("r0s")
        nc.vector.tensor_tensor(out=r0s[:, :], in0=crowsb[:, :], in1=dyf[:, :], op=AOP.add)
        x0g = t9("x0g")
        nc.vector.tensor_tensor(
            out=x0g[:, :],
            in0=bass.AP(cxwsb.tensor, 0, [[NK, 128], [0, EXT], [1, NK]]),
            in1=dxf[:, :].rearrange("p (r k) -> p r k", k=NK),
            op=AOP.add,
        )

        va = t9("va")
        vb = t9("vb")
        vv = t9("vv")
        p0t = t9("p0t")
        p1t = t9("p1t")
        q0t = t9("q0t")
        q1t = t9("q1t")

        def valid(src, slo, shi, dst):
            nc.vector.tensor_scalar(out=va[:, :], in0=src[:, :], scalar1=slo, scalar2=None, op0=AOP.is_ge)
            nc.vector.tensor_scalar(out=vb[:, :], in0=src[:, :], scalar1=shi, scalar2=None, op0=AOP.is_le)
            nc.vector.tensor_tensor(out=dst[:, :], in0=va[:, :], in1=vb[:, :], op=AOP.mult)

        # p0t = 2*(1-ty)*m2*vy0 ; p1t = 2*ty*m2*vy1
        valid(r0s, bndsb[:, 0:1], bndsb[:, 1:2], vv)
        nc.vector.tensor_scalar(out=p0t[:, :], in0=ty[:, :], scalar1=-2.0, scalar2=2.0, op0=AOP.mult, op1=AOP.add)
        nc.vector.tensor_tensor(out=p0t[:, :], in0=p0t[:, :], in1=m2[:, :], op=AOP.mult)
        nc.vector.tensor_tensor(out=p0t[:, :], in0=p0t[:, :], in1=vv[:, :], op=AOP.mult)
        valid(r0s, bndsb[:, 2:3], bndsb[:, 3:4], vv)
        nc.vector.tensor_scalar(out=p1t[:, :], in0=ty[:, :], scalar1=2.0, scalar2=None, op0=AOP.mult)
        nc.vector.tensor_tensor(out=p1t[:, :], in0=p1t[:, :], in1=m2[:, :], op=AOP.mult)
        nc.vector.tensor_tensor(out=p1t[:, :], in0=p1t[:, :], in1=vv[:, :], op=AOP.mult)
        # q0t = (1-tx)*vx0 ; q1t = tx*vx1
        valid(x0g, 0.0, 127.0, vv)
        nc.vector.tensor_scalar(out=q0t[:, :], in0=tx[:, :], scalar1=-1.0, scalar2=1.0, op0=AOP.mult, op1=AOP.add)
        nc.vector.tensor_tensor(out=q0t[:, :], in0=q0t[:, :], in1=vv[:, :], op=AOP.mult)
        valid(x0g, -1.0, 126.0, vv)
        nc.vector.tensor_tensor(out=q1t[:, :], in0=tx[:, :], in1=vv[:, :], op=AOP.mult)

        # paired-corner weight tiles, interleaved per (re,k):
        # u01[p, (re*9+k)*2 + j] = weight of corner (rj, x0); u23 same for x1
        u01 = sm.tile([128, 2 * N9], BF16, tag="u01", name="u01")
        u23 = sm.tile([128, 2 * N9], BF16, tag="u23", name="u23")
        nc.vector.tensor_tensor(
            out=bass.AP(u01.tensor, 0, [[2 * N9, 128], [2, N9]]),
            in0=p0t[:, :], in1=q0t[:, :], op=AOP.mult)
        nc.vector.tensor_tensor(
            out=bass.AP(u01.tensor, 1, [[2 * N9, 128], [2, N9]]),
            in0=p1t[:, :], in1=q0t[:, :], op=AOP.mult)
        nc.vector.tensor_tensor(
            out=bass.AP(u23.tensor, 0, [[2 * N9, 128], [2, N9]]),
            in0=p0t[:, :], in1=q1t[:, :], op=AOP.mult)
        nc.vector.tensor_tensor(
            out=bass.AP(u23.tensor, 1, [[2 * N9, 128], [2, N9]]),
            in0=p1t[:, :], in1=q1t[:, :], op=AOP.mult)

        # flat index = r0s*128 + x0g  (in-range by construction; clamp for safety)
        nc.vector.tensor_scalar(out=tmp[:, :], in0=r0s[:, :], scalar1=128.0, scalar2=None, op0=AOP.mult)
        nc.vector.tensor_tensor(out=tmp[:, :], in0=tmp[:, :], in1=x0g[:, :], op=AOP.add)
        nc.vector.tensor_scalar(out=tmp[:, :], in0=tmp[:, :], scalar1=0.0, scalar2=6800.0, op0=AOP.max, op1=AOP.min)
        nc.vector.tensor_copy(out=i32[:, :], in_=tmp[:, :])
        idx16 = sm.tile([128, N9], I16, tag="idx16", name="idx16")
        # i32 is (re,k) ordered; stream block b = Bg*9 + k*Rg + r per row-group
        for Bg, Rg in GRP:
            nc.vector.tensor_copy(
                out=bass.AP(idx16.tensor, Bg * 9, [[N9, 128], [Rg, NK], [1, Rg]]),
                in_=bass.AP(i32.tensor, Bg * 9, [[N9, 128], [1, NK], [9, Rg]]),
            )

        # bounce idx to DRAM (addr = lane*NB + b), then reload in the
        # dma_gather wrapped layout (replicated per 16-part group).
        # These DMAs ride the GpSimd queue (idle until the gathers).
        nc.sync.dma_start(
            out=bass.AP(idxd, 0, [[NB, 128], [1, NB]]),
            in_=idx16[:, :],
        )
        isbpre = cst.tile([128, 8 * NB], I16)
        for g in range(8):
            nc.sync.dma_start(
                out=bass.AP(isbpre.tensor, 16 * g * (8 * NB), [[8 * NB, 16], [NB, 8], [1, NB]]),
                in_=bass.AP(idxd, 0, [[NB, 16], [16 * NB, 8], [1, NB]]),
            )
        isb = cst.tile([128, NB * 8], I16)
        nc.vector.tensor_copy(
            out=bass.AP(isb.tensor, 0, [[8 * NB, 128], [8, NB], [1, 8]]),
            in_=bass.AP(isbpre.tensor, 0, [[8 * NB, 128], [1, NB], [NB, 8]]),
        )

        # ---------- Z planes (pair-interleaved bf16, 6-slot ring, batch-3 writes) ----------
        # slot layout per partition: [g<4: kin(2) x r(2) x 64 = 256e] x4, [g4: r(2) x 64]
        ZD = 6       # ring depth
        ZB = 3       # rows per zp write batch
        zpr = big.tile([128, ZD * 2432], BF16)
        ZFS = ZD * 2432  # zpr free size (partition stride)

        def zp_write_batch(r0, nrow, s0, eng):
            # rows [r0, r0+nrow) from zpr slots [s0, s0+nrow); one DMA per k-group
            for g in range(4):
                eng.dma_start(
                    out=bass.AP(zp, ZOFF[g] + r0 * W * 256,
                                [[256, 128], [128 * 256, nrow], [1, 256]]),
                    in_=bass.AP(zpr.tensor, s0 * 2432 + g * 512,
                                [[ZFS, 128], [2432, nrow], [1, 256]]),
                )
            eng.dma_start(
                out=bass.AP(zp, ZOFF[4] + r0 * W * 128, [[128, 128], [128 * 128, nrow], [1, 128]]),
                in_=bass.AP(zpr.tensor, s0 * 2432 + 2048, [[ZFS, 128], [2432, nrow], [1, 128]]),
            )

        nbatch = 0
        for ch in range(YR):
            pzt = pz.tile([128, 10 * C], F32, tag="pz", name="pzt", space="PSUM")
            nc.tensor.matmul(
                pzt[:, 0 : 8 * C],
                ysb[:, ch * W : (ch + 1) * W],
                wdcsb[:, 0 : 8 * C],
                start=True, stop=True,
            )
            nc.tensor.matmul(
                pzt[:, 8 * C : 10 * C],
                ysb[:, ch * W : (ch + 1) * W],
                wdcsb[:, 8 * C : 10 * C],
                start=True, stop=True,
            )
            s = ch % ZD
            # slot r0 (this row): all 10 slots in one strided copy
            nc.scalar.activation(
                bass.AP(zpr.tensor, s * 2432, [[ZFS, 128], [512, 5], [128, 2], [1, C]]),
                pzt[:, :].rearrange("p (g j o) -> p g j o", j=2, o=C),
                ACTF.Copy,
            )
            # slot r1 into previous row's slot (offset +64)
            if ch > 0:
                sp = (ch - 1) % ZD
                nc.vector.tensor_copy(
                    out=bass.AP(zpr.tensor, sp * 2432 + C, [[ZFS, 128], [512, 5], [128, 2], [1, C]]),
                    in_=pzt[:, :].rearrange("p (g j o) -> p g j o", j=2, o=C),
                )
                # batch-write fully completed rows [ch-ZB .. ch-1] when aligned
                if ch % ZB == 0 and ch >= ZB:
                    eng = nc.scalar if nbatch % 2 == 0 else nc.sync
                    zp_write_batch(ch - ZB, ZB, (ch - ZB) % ZD, eng)
                    nbatch += 1
        # tail rows: 54, 55 (55's r1 is garbage; never addressed by valid idx)
        zp_write_batch(YR - 2, 2, (YR - 2) % ZD, nc.scalar)

        # ---------- gather + combine + per-group tail ----------
        oslab = big.tile([C, EXT * WP], BF16)
        os3 = oslab[:, :].rearrange("p (r c) -> p r c", c=WP)
        nc.vector.memset(oslab[:, :], 0.0)
        t1 = big.tile([C, (EXT - 2) * WP], BF16)
        t13 = t1[:, :].rearrange("p (r c) -> p r c", c=WP)
        nc.vector.memset(t1[:, :], 0.0)

        for gi, (Bg, Rg) in enumerate(GRP):
            CNT = (Bg + Rg + 18) * 128  # zp blocks addressable by this group
            sacc01 = gp.tile([128, Rg * 128], BF16, tag="sacc01", name=f"s01_{gi}", bufs=2)
            sacc23 = gp.tile([128, Rg * 128], BF16, tag="sacc23", name=f"s23_{gi}", bufs=2)
            for k in range(NK):
                g4, kin = k // 2, k % 2
                blk = BLKG[g4]
                esz = blk + 128
                grun = gp.tile([128, Rg * 384], BF16, tag="grun", name="grun", bufs=4)
                gv = grun[:, : Rg * esz].rearrange("p (r e) -> p r e", e=esz)
                base = Bg * 9 + k * Rg  # stream block offset
                nc.gpsimd.dma_gather(
                    gv[:, :, :],
                    bass.AP(zp, ZOFF[g4] + kin * 128, [[blk, CNT], [1, esz]]),
                    isb[:, base * 8 : (base + Rg) * 8],
                    num_idxs=Rg * 128,
                    num_idxs_reg=Rg * 128,
                    elem_size=esz,
                    elem_step=blk,
                    queue_num=(gi * NK + k) % 4,
                )
                # paired-corner combine: (r0,r1)x64ch contiguous at x0 / x1
                ub = (Bg * 9 + k) * 2
                uap01 = bass.AP(u01.tensor, ub, [[2 * N9, 128], [18, Rg], [1, 2], [0, C]])
                uap23 = bass.AP(u23.tensor, ub, [[2 * N9, 128], [18, Rg], [1, 2], [0, C]])
                gva = bass.AP(grun.tensor, 0, [[Rg * 384, 128], [esz, Rg], [C, 2], [1, C]])
                gvb = bass.AP(grun.tensor, blk, [[Rg * 384, 128], [esz, Rg], [C, 2], [1, C]])
                if k == 0:
                    s01v = bass.AP(sacc01.tensor, 0, [[Rg * 128, 128], [128, Rg], [C, 2], [1, C]])
                    s23v = bass.AP(sacc23.tensor, 0, [[Rg * 128, 128], [128, Rg], [C, 2], [1, C]])
                    nc.vector.tensor_tensor(out=s01v, in0=gva, in1=uap01, op=AOP.mult)
                    nc.vector.tensor_tensor(out=s23v, in0=gvb, in1=uap23, op=AOP.mult)
                else:
                    tmpc = gp.tile([128, Rg * 128], BF16, tag="tmpc", name="tmpc", bufs=2)
                    tcv = bass.AP(tmpc.tensor, 0, [[Rg * 128, 128], [128, Rg], [C, 2], [1, C]])
                    nc.vector.tensor_tensor(out=tcv, in0=gva, in1=uap01, op=AOP.mult)
                    nc.vector.tensor_tensor(out=sacc01[:, :], in0=sacc01[:, :], in1=tmpc[:, :], op=AOP.add)
                    tmpd = gp.tile([128, Rg * 128], BF16, tag="tmpd", name="tmpd", bufs=2)
                    tdv = bass.AP(tmpd.tensor, 0, [[Rg * 128, 128], [128, Rg], [C, 2], [1, C]])
                    nc.vector.tensor_tensor(out=tdv, in0=gvb, in1=uap23, op=AOP.mult)
                    nc.vector.tensor_tensor(out=sacc23[:, :], in0=sacc23[:, :], in1=tmpd[:, :], op=AOP.add)
            # fold: accq = sacc01 + sacc23 (f32), then fold the (r0,r1) pair
            accq = gp.tile([128, Rg * 128], F32, tag="accq", name=f"accq{gi}", bufs=2)
            nc.vector.tensor_tensor(out=accq[:, :], in0=sacc01[:, :], in1=sacc23[:, :], op=AOP.add)
            acc = gp.tile([128, Rg * C], F32, tag="acc", name=f"acc{gi}", bufs=2)
            nc.vector.tensor_tensor(
                out=acc[:, :].rearrange("p (r o) -> p r o", o=C),
                in0=bass.AP(accq.tensor, 0, [[Rg * 128, 128], [128, Rg], [1, C]]),
                in1=bass.AP(accq.tensor, C, [[Rg * 128, 128], [128, Rg], [1, C]]),
                op=AOP.add,
            )
            acc3 = acc[:, :].rearrange("p (r o) -> p r o", o=C)

            # transpose back (batches of 4 rows) + bdc + x0 residual -> oslab
            for rb in range(Rg // 4):
                ptb = pt.tile([128, 512], F32, tag="tr", name="ptb", space="PSUM")
                for i in range(4):
                    nc.tensor.transpose(
                        ptb[0:C, i * W : (i + 1) * W],
                        acc3[:, rb * 4 + i, :],
                        ident[:, :],
                    )
                tdc = gp.tile([C, 512], F32, tag="tdc", bufs=2)
                nc.vector.tensor_scalar(out=tdc[:, :], in0=ptb[0:C, :], scalar1=bdcsb[:, :], scalar2=None, op0=AOP.add)
                re0 = Bg + rb * 4
                nc.vector.tensor_tensor(
                    out=bass.AP(os3.tensor, re0 * WP + 1, [[EXT * WP, C], [WP, 4], [1, W]]),
                    in0=tdc[:, :].rearrange("p (r c) -> p r c", c=W),
                    in1=bass.AP(x0y3.tensor, (re0 + 1) * WP + 1, [[XR * WP, C], [WP, 4], [1, W]]),
                    op=AOP.add,
                )
            # zero out-of-image rows for this group
            nc.vector.tensor_tensor(
                out=os3[:, Bg : Bg + Rg, :],
                in0=os3[:, Bg : Bg + Rg, :],
                in1=bass.AP(m36sb.tensor, Bg, [[EXT, C], [1, Rg], [0, WP]]),
                op=AOP.mult,
            )

            # conv1 chunk enabled by this group (+ lrelu + m34 mask)
            r0c, nr_h = C1CH[gi]
            pc1 = ps.tile([C, 8 * W], F32, tag="mm", name="pc1", space="PSUM")[:, : nr_h * W]
            for k in range(NK):
                ki, kj = k // 3, k % 3
                n = 0
                while n < nr_h:
                    nr = min(4, nr_h - n)
                    re = r0c + n
                    nc.tensor.matmul(
                        pc1[:, n * W : (n + nr) * W],
                        w1sb[:, k * C : (k + 1) * C],
                        bass.AP(os3.tensor, (re - 1 + ki) * WP + kj, [[EXT * WP, C], [WP, nr], [1, W]]),
                        start=(k == 0), stop=(k == NK - 1),
                    )
                    n += nr
            tl = gp.tile([C, 8 * W], F32, tag="tl", name="tl", bufs=2)
            nc.vector.tensor_scalar(out=tl[:, : nr_h * W], in0=pc1[:, :], scalar1=b1sb[:, :], scalar2=None, op0=AOP.add)
            # t1 row (T1 coords = EXT row - 1) = lrelu * rowmask
            nc.vector.scalar_tensor_tensor(
                out=bass.AP(t13.tensor, (r0c - 1) * WP + 1, [[(EXT - 2) * WP, C], [WP, nr_h], [1, W]]),
                in0=tl[:, : nr_h * W].rearrange("p (r c) -> p r c", c=W),
                scalar=0.2,
                in1=tl[:, : nr_h * W].rearrange("p (r c) -> p r c", c=W),
                op0=AOP.mult,
                op1=AOP.max,
            )
            nc.vector.tensor_tensor(
                out=t13[:, r0c - 1 : r0c - 1 + nr_h, :],
                in0=t13[:, r0c - 1 : r0c - 1 + nr_h, :],
                in1=bass.AP(m34sb.tensor, r0c - 1, [[EXT - 2, C], [1, nr_h], [0, WP]]),
                op=AOP.mult,
            )

            # conv2 chunk + residual + store
            o0, nr_h = C2CH[gi]
            pc2 = ps.tile([C, 8 * W], F32, tag="mm", name="pc2", space="PSUM")[:, : nr_h * W]
            for k in range(NK):
                ki, kj = k // 3, k % 3
                n = 0
                while n < nr_h:
                    nr = min(4, nr_h - n)
                    # conv2 out row o reads t1 rows (o+ki) in T1 coords
                    nc.tensor.matmul(
                        pc2[:, n * W : (n + nr) * W],
                        w2sb[:, k * C : (k + 1) * C],
                        bass.AP(t13.tensor, (o0 + n + ki) * WP + kj, [[(EXT - 2) * WP, C], [WP, nr], [1, W]]),
                        start=(k == 0), stop=(k == NK - 1),
                    )
                    n += nr
            tf = gp.tile([C, 8 * W], F32, tag="tf", name="tf", bufs=2)
            nc.vector.tensor_scalar(out=tf[:, : nr_h * W], in0=pc2[:, :], scalar1=b2sb[:, :], scalar2=None, op0=AOP.add)
            nc.vector.tensor_tensor(
                out=tf[:, : nr_h * W].rearrange("p (r c) -> p r c", c=W),
                in0=tf[:, : nr_h * W].rearrange("p (r c) -> p r c", c=W),
                in1=bass.AP(os3.tensor, (o0 + 2) * WP + 1, [[EXT * WP, C], [WP, nr_h], [1, W]]),
                op=AOP.add,
            )
            nc.sync.dma_start(
                out=outp[:, o0 * W : (o0 + nr_h) * W], in_=tf[:, : nr_h * W]
            )

    nc.finalize()
    return nc


# ---------------- host side ----------------

_NC_CACHE = None


def _get_nc():
    global _NC_CACHE
    if _NC_CACHE is None:
        _NC_CACHE = build_nc()
    return _NC_CACHE


def _prep_core(inputs, b, q):
    G0 = 32 * q
    x = inputs["x"][b]  # [64, 128, 128]
    y = inputs["y"][b]

    def slab(img, lo, rows):
        out = np.zeros((C, rows, W), np.float32)
        for i in range(rows):
            g = lo + i
            if 0 <= g < 128:
                out[:, i, :] = img[:, g, :]
        return out

    import ml_dtypes
    bf = ml_dtypes.bfloat16
    xs = slab(x, G0 - 3, XR).reshape(C, XR * W).astype(bf)
    ysl = slab(y, G0 - 12, YR).reshape(C, YR * W).astype(bf)

    w0t = inputs["w0"][:, :, 0, 0].T.copy().astype(bf)  # [c, o]
    womt = (np.transpose(inputs["w_om"], (2, 3, 1, 0)).reshape(NK, 128, 27).reshape(NK * 128, 27).copy()).astype(bf)
    wdct = (np.transpose(inputs["w_dc"], (2, 3, 1, 0)).reshape(NK, C, C).reshape(NK * C, C).copy()).astype(bf)
    w1t = (np.transpose(inputs["w1"], (2, 3, 1, 0)).reshape(NK, C, C).reshape(NK * C, C).copy()).astype(bf)
    w2t = (np.transpose(inputs["w2"], (2, 3, 1, 0)).reshape(NK, C, C).reshape(NK * C, C).copy()).astype(bf)

    lo = 12.0 - G0
    hi = 139.0 - G0
    bnd = np.tile(np.array([[lo, hi, lo - 1.0, hi - 1.0]], np.float32), (128, 1))

    re_idx = np.arange(EXT)[:, None]
    ki = (np.arange(NK) // 3)[None, :]
    kj = (np.arange(NK) % 3)[None, :]
    crow_row = (re_idx + ki + 9).astype(np.float32).reshape(1, EXT * NK)
    crow = np.tile(crow_row, (128, 1))
    wv = np.arange(128)[:, None].astype(np.float32)
    cxw = (wv - 1.0 + kj.astype(np.float32))  # [128, 9]

    def rowmask(lo_r, rows):
        g = lo_r + np.arange(rows)
        m = ((g >= 0) & (g < 128)).astype(np.float32)
        return np.tile(m[None, :], (C, 1))

    return {
        "xs": xs,
        "ys": ysl,
        "w0t": w0t,
        "b0": inputs["b0"].reshape(C, 1).astype(np.float32),
        "womt": womt,
        "bom": inputs["b_om"].reshape(27, 1).astype(np.float32),
        "wdct": wdct,
        "bdc": inputs["b_dc"].reshape(C, 1).astype(np.float32),
        "w1t": w1t,
        "b1": inputs["b1"].reshape(C, 1).astype(np.float32),
        "w2t": w2t,
        "b2": inputs["b2"].reshape(C, 1).astype(np.float32),
        "bnd": bnd,
        "crow": crow,
        "cxw": cxw.astype(np.float32),
        "m38": rowmask(G0 - 3, XR),
        "m36": rowmask(G0 - 2, EXT),
        "m34": rowmask(G0 - 1, EXT - 2),
    }


def make_in_maps(inputs):
    inputs = {k: np.asarray(v, np.float32) for k, v in inputs.items()}
    return [_prep_core(inputs, core // 4, core % 4) for core in range(8)]


def kernel(**inputs):
    from concourse.bass_utils import run_bass_kernel_spmd

    nc = _get_nc()
    in_maps = make_in_maps(inputs)
    res = run_bass_kernel_spmd(nc, in_maps, core_ids=list(range(8)))
    out = np.zeros((2, C, 128, W), np.float32)
    for core in range(8):
        b, q = core // 4, core % 4
        out[b, :, 32 * q : 32 * q + 32, :] = res.results[core]["out"].reshape(C, OH, W)
    return out
